# revision 4
# baseline (speedup 1.0000x reference)
"""Trainium2 Bass kernel for nn_AttentionModule (B=8, C=128, H=W=256).

out[b,c] = softmax((W1 x_b + b1)[c] @ ((W2 x_b + b2)[c])^T) @ (W2 x_b + b2)[c] + x_b[c]

Sharding: data-parallel over batch B across the 8 NeuronCores (1 batch each);
weights replicated. Each core runs an identical single-core NEFF.

Per-core plan (all fp32):
  Phase A (x streamed in 4 passes, one per 32-channel group):
    trick-GEMM per (h, w-chunk): out[w,128] (PSUM) with stationary
    lhsT = x[:, h, wchunk] (c-on-partition) and moving rhs = [W1^T|W2^T]
    group columns -> produces q^T/k^T directly in [w, (c,h)] layout,
    i.e. per-channel 256x256 matrices with w on partitions -- no separate
    transpose pass for Q/K. Evac PSUM->SBUF: q on DVE (+b1 bias pattern
    add), k on ACT (plain copy; see bias algebra below).
  Phase B per channel:
    scores[h,g] = sum_w qT[w,h] kT[w,g]    (2 h-tiles x 2 w-chunk accum)
    softmax rows: -max (DVE reduce, negate) -> exp on ACT (bias=-max,
    accum_out=l) -> P *= 1/l (DVE per-partition scalar)
    PE-transpose P -> attnT [g,h]; PE-transpose kT -> k_nat [g,w]
    out[h,w] = sum_g attnT^T P... = matmul(lhsT=attnT, rhs=k_nat), 2 g-chunk accum
    residual: out += x_c + b2[c]  (one DVE scalar_tensor_tensor)

Bias algebra: k is kept UNBIASED on chip. The b2 shift adds a per-row
constant to the scores (softmax-invariant) and, since softmax rows sum to
1, contributes exactly +b2[c] to the output -- folded into the residual.

Container workarounds (see _apply_tile_patches):
  - walrus here encodes at most one sem wait per instruction -> split.
  - EVSEM butterfly barrier hangs at runtime -> NRT pseudo barrier.
  - sem_clear/dma_reset hang -> skipped (one execution per model load).
  - HWDGE (nc.sync) DMAs hang under Tile -> all DMAs on gpsimd (SWDGE).
"""

import sys

if '/opt/trn_rl_repo' not in sys.path:
    sys.path.insert(0, '/opt/trn_rl_repo')

import numpy as np

B, C, H, W = 8, 128, 256, 256
G = 32            # channels per group
NG = C // G       # 4 groups / x passes
HB = 8            # h rows per Phase-A step (fills one [128,1024] PSUM pair)
N_CORES = 8
HW_ELEMS = H * W

_patched = False


def _apply_tile_patches():
    global _patched
    if _patched:
        return
    _patched = True
    import concourse.tile as tile
    from concourse.vector_clock import ScopedClock

    def _drain_and_barrier(self, tick_clock, wait_clock):
        nc = self.nc
        drain_inst = nc.sync.drain()
        wait_clock.add_sem_waits(
            drain_inst.ins, ScopedClock({None: tick_clock.global_clock})
        )
        nc._nrt_pseudo_barrier()
        assert self.sems is not None
        popped = nc._tile_sem_poison_stack.pop()
        assert popped is self._sem_poison
        # No sem_clear / dma_reset: RANGE_CLEAR and DMA_RESET hang on this
        # runtime. Sound because every kernel() call loads a fresh
        # executable (NRT zeroes semaphores at load).

    tile.TileContext._drain_and_barrier = _drain_and_barrier


def _split_multi_waits(nc):
    from concourse import mybir
    n = 0
    for f in nc.m.functions:
        for blk in f.blocks:
            insts = list(blk.instructions)
            out = []
            changed = False
            for inst in insts:
                si = getattr(inst, "sync_info", None)
                if si is not None and len(si.on_wait) > 1:
                    waits = list(si.on_wait)
                    for i, w in enumerate(waits[:-1]):
                        nop = mybir.InstNoOp(
                            name=f"{inst.name}_wsplit{i}", ins=[], outs=[])
                        nop.engine = inst.engine
                        nop.sync_info = mybir.SyncInfo(on_wait=[w], on_update=[])
                        out.append(nop)
                        n += 1
                    inst.sync_info = mybir.SyncInfo(
                        on_wait=[waits[-1]], on_update=list(si.on_update))
                    changed = True
                out.append(inst)
            if changed:
                blk.instructions = out
    return n


def build_program(patch=True):
    """Build the single-core Bass program. Returns nc."""
    if patch:
        _apply_tile_patches()
    import concourse.bass as bass
    import concourse.tile as tile
    from concourse import mybir
    from contextlib import ExitStack

    f32 = mybir.dt.float32
    AF = mybir.ActivationFunctionType
    ALU = mybir.AluOpType
    AX = mybir.AxisListType

    nc = bass.Bass("TRN2", target_bir_lowering=False, debug=False, num_devices=1)
    x_t = nc.dram_tensor("x", [C, H, W], f32, kind="ExternalInput")
    wcat_t = nc.dram_tensor("wcat", [C, 2 * C], f32, kind="ExternalInput")
    biasq_t = nc.dram_tensor("biasq", [128, NG * 2 * G * HB], f32,
                             kind="ExternalInput")  # [g][i(HB)][wc(2)][c(G)] repl.
    b2b_t = nc.dram_tensor("b2b", [128, C], f32, kind="ExternalInput")
    ident_t = nc.dram_tensor("ident", [128, 128], f32, kind="ExternalInput")
    out_t = nc.dram_tensor("out", [C, H, W], f32, kind="ExternalOutput")

    x_ap = x_t.ap()       # [128(c), 256, 256]
    out_h = out_t
    GRP = 2 * G * HB      # 512 bias-pattern cols per group

    def dram_hslab(tensor, c, ht):
        # [h(128 partitions), w] slab of [C,H,W] dram tensor for channel c
        return bass.AP(tensor.ap().tensor, c * HW_ELEMS + ht * 128 * W,
                       [[W, 128], [1, W]])

    with tile.TileContext(nc) as tc, ExitStack() as ctx:
        consts = ctx.enter_context(tc.tile_pool(name="consts", bufs=1))
        gq = ctx.enter_context(tc.tile_pool(name="gq", bufs=1))
        gk = ctx.enter_context(tc.tile_pool(name="gk", bufs=1))
        xpool = ctx.enter_context(tc.tile_pool(name="xpool", bufs=3))
        ppool = ctx.enter_context(tc.tile_pool(name="ppool", bufs=3))
        atpool = ctx.enter_context(tc.tile_pool(name="atpool", bufs=4))
        knpool = ctx.enter_context(tc.tile_pool(name="knpool", bufs=4))
        opool = ctx.enter_context(tc.tile_pool(name="opool", bufs=3))
        xrpool = ctx.enter_context(tc.tile_pool(name="xrpool", bufs=3))
        stats = ctx.enter_context(tc.tile_pool(name="stats", bufs=4))
        psA = ctx.enter_context(tc.tile_pool(name="psA", bufs=2, space="PSUM"))
        ps256 = ctx.enter_context(tc.tile_pool(name="ps256", bufs=4, space="PSUM"))

        wcat_sb = consts.tile([128, 2 * C], f32)
        nc.gpsimd.dma_start(out=wcat_sb[:], in_=wcat_t.ap())
        ident_sb = consts.tile([128, 128], f32)
        nc.gpsimd.dma_start(out=ident_sb[:], in_=ident_t.ap())
        b2b_sb = consts.tile([128, C], f32)
        nc.gpsimd.dma_start(out=b2b_sb[:], in_=b2b_t.ap())

        for g in range(NG):
            biasq_sb = consts.tile([128, GRP], f32, tag="biasq_sb")
            nc.gpsimd.dma_start(out=biasq_sb[:],
                                in_=biasq_t.ap()[:, g * GRP:(g + 1) * GRP])

            # group-resident qT/kT: [128(w), wc(2) x c(G) x h(H)]
            qT = gq.tile([128, 2 * G * H], f32, tag="qT")
            kT = gk.tile([128, 2 * G * H], f32, tag="kT")

            # ---------------- Phase A ----------------
            for hb in range(0, H, HB):
                xt = xpool.tile([128, HB * W], f32, tag="xt")
                nc.gpsimd.dma_start(
                    out=xt[:].rearrange("p (a b) -> p a b", a=HB),
                    in_=x_ap[:, hb:hb + HB, :])
                # PSUM [128, HB*128]: layout [i(HB)][wc(2)][t(2)][c(G)]
                ps = psA.tile([128, HB * 128], f32, tag="psA")
                for i in range(HB):
                    for wc in range(2):
                        nc.tensor.matmul(
                            out=ps[:, i * 128 + wc * 64: i * 128 + wc * 64 + 64],
                            lhsT=xt[:, i * W + wc * 128: i * W + wc * 128 + 128],
                            rhs=wcat_sb[:, g * 64:(g + 1) * 64],
                            start=(wc == 0 and i in (0, HB // 2)),
                            stop=(wc == 1 and i in (HB // 2 - 1, HB - 1)),
                        )
                # evac q (DVE, + b1 pattern) ; k (ACT, plain copy)
                # in dims (i, wc, c): psA strides (128, 64, 1)
                ps_q = bass.AP(ps[:].tensor, ps[:].offset,
                               [ps[:].ap[0], [128, HB], [64, 2], [1, G]])
                ps_k = bass.AP(ps[:].tensor, ps[:].offset + 32,
                               [ps[:].ap[0], [128, HB], [64, 2], [1, G]])
                bq = bass.AP(biasq_sb[:].tensor, biasq_sb[:].offset,
                             [biasq_sb[:].ap[0], [2 * G, HB], [G, 2], [1, G]])
                # out dims (i, wc, c): qT strides (1, G*H, H), offset hb
                q_out = bass.AP(qT[:].tensor, qT[:].offset + hb,
                                [qT[:].ap[0], [1, HB], [G * H, 2], [H, G]])
                k_out = bass.AP(kT[:].tensor, kT[:].offset + hb,
                                [kT[:].ap[0], [1, HB], [G * H, 2], [H, G]])
                nc.vector.tensor_add(q_out, ps_q, bq)
                nc.scalar.activation(k_out, ps_k, AF.Copy)

            # ---------------- Phase B ----------------
            for cl in range(G):
                c = g * G + cl
                q0 = qT[:, cl * H: cl * H + H]            # wc=0 [w128, h256]
                q1 = qT[:, G * H + cl * H: G * H + cl * H + H]
                k0 = kT[:, cl * H: cl * H + H]
                k1 = kT[:, G * H + cl * H: G * H + cl * H + H]

                negmax = stats.tile([128, 2], f32, tag="negmax")
                lsum = stats.tile([128, 2], f32, tag="lsum")
                rinv = stats.tile([128, 2], f32, tag="rinv")
                P = []
                for ht in range(2):
                    ss = ps256.tile([128, 256], f32, tag="ps256")
                    nc.tensor.matmul(out=ss[:], lhsT=q0[:, ht * 128:(ht + 1) * 128],
                                     rhs=k0, start=True, stop=False)
                    nc.tensor.matmul(out=ss[:], lhsT=q1[:, ht * 128:(ht + 1) * 128],
                                     rhs=k1, start=False, stop=True)
                    nc.vector.tensor_reduce(
                        out=negmax[:, ht:ht + 1], in_=ss[:], axis=AX.X,
                        op=ALU.max, negate=True)
                    p = ppool.tile([128, 256], f32, tag="P")
                    nc.scalar.activation(p[:], ss[:], AF.Exp,
                                         bias=negmax[:, ht:ht + 1], scale=1.0,
                                         accum_out=lsum[:, ht:ht + 1])
                    P.append(p)
                nc.vector.reciprocal(rinv[:], lsum[:])
                for ht in range(2):
                    nc.vector.tensor_scalar_mul(P[ht][:], P[ht][:],
                                                rinv[:, ht:ht + 1])

                # transposes: attnT per gc (from P), k_nat per gc (from kT)
                at_sb = []
                kn_sb = []
                for gc in range(2):
                    pt = ps256.tile([128, 256], f32, tag="ps256")
                    for ht in range(2):
                        nc.tensor.matmul(
                            out=pt[:, ht * 128:(ht + 1) * 128],
                            lhsT=P[ht][:, gc * 128:(gc + 1) * 128],
                            rhs=ident_sb[:], is_transpose=True,
                            start=(ht == 0), stop=(ht == 1))
                    a = atpool.tile([128, 256], f32, tag="attnT")
                    nc.scalar.activation(a[:], pt[:], AF.Copy)
                    at_sb.append(a)

                    pk = ps256.tile([128, 256], f32, tag="ps256")
                    for wc, ksrc in ((0, k0), (1, k1)):
                        nc.tensor.matmul(
                            out=pk[:, wc * 128:(wc + 1) * 128],
                            lhsT=ksrc[:, gc * 128:(gc + 1) * 128],
                            rhs=ident_sb[:], is_transpose=True,
                            start=(wc == 0), stop=(wc == 1))
                    kn = knpool.tile([128, 256], f32, tag="knat")
                    nc.scalar.activation(kn[:], pk[:], AF.Copy)
                    kn_sb.append(kn)

                for ht in range(2):
                    po = ps256.tile([128, 256], f32, tag="ps256")
                    for gc in range(2):
                        nc.tensor.matmul(
                            out=po[:], lhsT=at_sb[gc][:, ht * 128:(ht + 1) * 128],
                            rhs=kn_sb[gc][:], start=(gc == 0), stop=(gc == 1))
                    xr = xrpool.tile([128, 256], f32, tag="xr")
                    nc.gpsimd.dma_start(out=xr[:], in_=dram_hslab(x_t, c, ht))
                    ob = opool.tile([128, 256], f32, tag="ob")
                    # ob = (po + b2[c]) + xr
                    nc.vector.scalar_tensor_tensor(
                        out=ob[:], in0=po[:], scalar=b2b_sb[:, c:c + 1],
                        in1=xr[:], op0=ALU.add, op1=ALU.add)
                    nc.gpsimd.dma_start(out=dram_hslab(out_h, c, ht), in_=ob[:])
    return nc


def _host_inputs(x_b, W1, b1, W2, b2):
    wcat = np.empty((C, 2 * C), np.float32)
    for g in range(NG):
        for t, Wm in ((0, W1), (1, W2)):
            for cl in range(G):
                wcat[:, g * 64 + t * 32 + cl] = Wm[g * G + cl, :]
    biasq = np.empty((128, NG * 2 * G * HB), np.float32)
    for g in range(NG):
        pat = np.empty((HB, 2, G), np.float32)
        pat[:, :, :] = b1[g * G:(g + 1) * G][None, None, :]
        biasq[:, g * 512:(g + 1) * 512] = pat.reshape(-1)[None, :]
    b2b = np.broadcast_to(b2[None, :], (128, C)).copy()
    ident = np.eye(128, dtype=np.float32)
    return {"x": np.ascontiguousarray(x_b, np.float32), "wcat": wcat,
            "biasq": biasq, "b2b": b2b, "ident": ident}


def kernel(x, W1, b1, W2, b2, _trace=False):
    import concourse.bass_utils as bass_utils

    nc = build_program(patch=True)
    nsplit = _split_multi_waits(nc)

    in_maps = [_host_inputs(x[b], W1, b1, W2, b2) for b in range(B)]
    kw = {}
    if _trace:
        kw = dict(trace=True, trace_cores=[0])
    res = bass_utils.run_bass_kernel_spmd(
        nc, in_maps, core_ids=list(range(N_CORES)), **kw)
    out = np.stack([res.results[b]["out"] for b in range(B)], axis=0)
    if _trace:
        kernel._last_results = res
    return out
